# revision 37
# baseline (speedup 1.0000x reference)
"""Trainium2 Bass kernel for nn_AttentionBlock (B=8,S=1024,E=1024,H=16,FF=4096).

Strategy: pure data-parallel over batch — each of the 8 NeuronCores runs the
full attention block on one [S,E] slice. No collectives.

Per-core layout convention: every activation lives feature-major ("T" =
[feature, token]) in SBUF so that each matmul consumes the previous output
directly (weights are pre-transposed on the host; the TensorEngine computes
lhsT.T @ rhs). All f32 matmul operands are stored as float32r, which the PE
runs single-pass (fp32_mode=HIGH, ~1 cyc/row at 512-wide moving dim) instead
of the 4-pass full-precision fp32 mode. Softmax uses a constant logit shift
(no max pass — logits are bounded well inside fp32 exp range for this scale),
the denominator comes from a ones-column appended to V, and normalization is
applied to the [64, S] context rows rather than the [S, S] score matrix.

Attention epilogue keeps the PE stream free of round-trips: the reciprocal
runs on DVE straight out of PSUM, the across-partition broadcast runs on the
(otherwise idle) GPSIMD engine, and the normalize multiply on DVE — no PE
broadcast matmuls, so PSUM fits scores (2x[128,S]) + 4 ctx accumulators.

LayerNorm reduces over the partition axis via (1/E)-scaled all-ones matmuls
whose [128, S] PSUM output doubles as the partition-broadcast of the per-token
mean / second moment; those matmuls are interleaved one-et-deferred into the
producing GEMM's PE stream so the PE never waits on the residual adds. The
gamma/beta application rides the scalar engine (Identity activation with
per-partition scale/bias).

SBUF slot reuse (pool release is LIFO, so lifetimes must nest): the ctx tiles
take over the dead Q tiles' slots (heads run in pairs so Q tile j is fully
dead first), and residual/LN/FFN epilogues run in place in the x tiles, which
successively hold x -> hpre -> h -> y -> out.
"""
import math
import numpy as np
import ml_dtypes

import concourse.bass as bass
import concourse.mybir as mybir
from concourse.tile import TileContext
from concourse.bass_utils import run_bass_kernel_spmd
from concourse.vector_clock import ScopedClock, VectorClock


def _split_drain_and_barrier(self, tick_clock, wait_clock):
    """Replacement for TileContext._drain_and_barrier: this walrus build
    allows only ONE sync-wait command on NoOp/Drain instructions, so the
    end-of-kernel drain's per-processor waits are split across single-wait
    SP nops (the SP sequencer is in-order, so by the drain every condition
    holds)."""
    gc = tick_clock.global_clock
    n = len(gc)
    for i in range(n):
        if gc[i] <= 0:
            continue
        vc = VectorClock([gc[j] if j == i else 0 for j in range(n)])
        nop_inst = self.nc.sync.nop()
        wait_clock.add_sem_waits(nop_inst.ins, ScopedClock({None: vc}))
    self.nc.sync.drain()
    self.nc.all_engine_barrier()
    assert self.sems is not None
    popped = self.nc._tile_sem_poison_stack.pop()
    assert popped is self._sem_poison
    self.nc.clear_and_free_semaphores(list(self.sems.allocated().values()))
    self.nc.all_engine_barrier()


TileContext._drain_and_barrier = _split_drain_and_barrier


def _split_multi_waits(nc):
    """This walrus build supports a single sync-wait command per instruction.
    Hoist all but one wait of any instruction onto fresh single-wait NoOps on
    the same engine, inserted immediately before it (engine queues are
    in-order, so the semantics are identical)."""
    ctr = 0

    def walk(blocks):
        nonlocal ctr
        for b in blocks:
            il = b.instructions
            i = 0
            while i < len(il):
                inst = il[i]
                si = inst.sync_info
                waits = list(si.on_wait) if (si is not None and si.on_wait) else []
                if len(waits) > 1:
                    for w in waits[:-1]:
                        ctr += 1
                        nop = mybir.InstNoOp(
                            name=f"I-wsplit-{ctr}", engine=inst.engine,
                            ins=[], outs=[])
                        nop.sync_info = mybir.SyncInfo(on_wait=[w], on_update=[])
                        nc.register_instruction(nop, overwrite=True)
                        il.insert(i, nop)
                        i += 1
                    inst.sync_info = mybir.SyncInfo(
                        on_wait=[waits[-1]],
                        on_update=list(si.on_update) if si.on_update else [])
                i += 1
            sub = getattr(b, "blocks", None)
            if sub:
                walk(sub)

    for f in nc.m.functions:
        walk(f.blocks)

F32 = mybir.dt.float32
F32R = mybir.dt.float32r
BF16 = mybir.dt.bfloat16
AF = mybir.ActivationFunctionType
OP = mybir.AluOpType

B, E, H, FF = 8, 1024, 16, 4096
HD = E // H  # 64
N_DOM = 1024
SCALE = math.sqrt(1.0 / HD) * 2.0 * math.log(N_DOM)  # 1.73287
SHIFT = -40.0  # constant logit shift inside exp; see module docstring
LN_EPS = 1e-5
NCORES = 8

# Per-matmul-group compute dtype for f32-stored operands: F32R (single-pass
# "HIGH" mode, ~1 cyc/row at 512-wide moving dim, bf16-grade multiply
# precision with f32 accumulate) or F32 (4-pass, exact, 4x slower).
DEFAULT_CFG = {
    "main": F32R,
    "scores": F32R,
    "outp": F32R,
}


def build_bass(S=1024, cfg=None):
    cfg = dict(DEFAULT_CFG, **(cfg or {}))
    MDT = cfg["main"]      # dtype of x/h/y tiles, qkv+ffn1 weights, LN ones
    SDT = cfg["scores"]    # dtype of Q/K tiles
    ODT = cfg["outp"]      # dtype of ctx tiles + out-proj weights
    ET = E // 128          # 8 e-tiles
    ST = S // 128          # s-tiles
    SH = S // 512          # 512-wide column halves
    FT1 = FF // 128        # 32 f-tiles for FFN hidden

    nc = bass.Bass()
    xT_d = nc.declare_dram_parameter("xT", [E, S], MDT, isOutput=False)
    wqkT_d = nc.declare_dram_parameter("wqkT", [E, 2 * E], MDT, isOutput=False)
    wvT_d = nc.declare_dram_parameter("wvT", [E, E], MDT, isOutput=False)
    woT_d = nc.declare_dram_parameter("woT", [E, E], ODT, isOutput=False)
    w1T_d = nc.declare_dram_parameter("w1T", [E, FF], MDT, isOutput=False)
    w2T_d = nc.declare_dram_parameter("w2T", [FF, E], BF16, isOutput=False)
    b1_d = nc.declare_dram_parameter("b1t", [128, FF // 128], F32, isOutput=False)
    b2_d = nc.declare_dram_parameter("b2t", [128, ET], F32, isOutput=False)
    g1_d = nc.declare_dram_parameter("g1t", [128, ET], F32, isOutput=False)
    be1_d = nc.declare_dram_parameter("be1t", [128, ET], F32, isOutput=False)
    g2_d = nc.declare_dram_parameter("g2t", [128, ET], F32, isOutput=False)
    be2_d = nc.declare_dram_parameter("be2t", [128, ET], F32, isOutput=False)
    ones_d = nc.declare_dram_parameter("ones128", [128, 128], MDT, isOutput=False)
    onesr_d = nc.declare_dram_parameter("onesr", [1, 64], SDT, isOutput=False)
    out_d = nc.declare_dram_parameter("outT", [E, S], MDT, isOutput=True)

    with TileContext(nc) as tc:
        cpool = tc.alloc_tile_pool(name="consts", bufs=1)
        xp = tc.alloc_tile_pool(name="xp", bufs=1)

        # ones128 holds 1/E so the LN sum-matmuls directly produce the
        # partition-broadcast mean / second moment in PSUM.
        ones128 = cpool.tile([128, 128], MDT, tag="ones128")
        nc.sync.dma_start(out=ones128[:], in_=ones_d[:])
        ones_r = cpool.tile([1, 64], SDT, tag="ones_r")
        nc.sync.dma_start(out=ones_r[:], in_=onesr_d[:])
        shift_ap = cpool.tile([128, 1], F32, tag="shift")
        nc.vector.memset(shift_ap[:], SHIFT)
        eps_ap = cpool.tile([128, 1], F32, tag="eps")
        nc.vector.memset(eps_ap[:], LN_EPS)
        b1s = cpool.tile([128, FF // 128], F32, tag="b1s")
        nc.sync.dma_start(out=b1s[:], in_=b1_d[:])
        b2s = cpool.tile([128, ET], F32, tag="b2s")
        nc.sync.dma_start(out=b2s[:], in_=b2_d[:])
        g1s = cpool.tile([128, ET], F32, tag="g1s")
        nc.sync.dma_start(out=g1s[:], in_=g1_d[:])
        be1s = cpool.tile([128, ET], F32, tag="be1s")
        nc.sync.dma_start(out=be1s[:], in_=be1_d[:])
        g2s = cpool.tile([128, ET], F32, tag="g2s")
        nc.sync.dma_start(out=g2s[:], in_=g2_d[:])
        be2s = cpool.tile([128, ET], F32, tag="be2s")
        nc.sync.dma_start(out=be2s[:], in_=be2_d[:])

        x_sb = []
        for et in range(ET):
            t = xp.tile([128, S], MDT, tag=f"x{et}", name=f"x{et}")
            nc.sync.dma_start(out=t[:], in_=xT_d[et * 128:(et + 1) * 128, :])
            x_sb.append(t)

        # ---------------- Stage A: QKV projection ----------------
        qkp = tc.alloc_tile_pool(name="qk", bufs=1)
        vap = tc.alloc_tile_pool(name="va", bufs=1)
        qk_sb = [qkp.tile([128, S], SDT, tag=f"qk{j}", name=f"qk{j}")
                 for j in range(2 * ET)]
        v_sb = [vap.tile([128, 16 * 65], BF16, tag=f"va{st}", name=f"va{st}")
                for st in range(ST)]
        wqk_r = wqkT_d.rearrange("(a p) f -> p a f", p=128)
        wsp = tc.alloc_tile_pool(name="wslabA", bufs=3)
        wvp = tc.alloc_tile_pool(name="wv", bufs=1)
        psA = tc.alloc_tile_pool(name="psA", bufs=4, space="PSUM")
        for ftile in range(2 * ET):
            slab = wsp.tile([128, ET * 128], MDT, tag="wslabA")
            nc.sync.dma_start(
                out=slab[:].rearrange("p (a f) -> p a f", a=ET),
                in_=wqk_r[:, :, ftile * 128:(ftile + 1) * 128],
            )
            for sh in range(SH):
                ps = psA.tile([128, 512], F32, tag="psA")
                for et in range(ET):
                    nc.tensor.matmul(
                        ps[:],
                        slab[:, et * 128:(et + 1) * 128],
                        x_sb[et][:, sh * 512:(sh + 1) * 512],
                        start=(et == 0), stop=(et == ET - 1),
                    )
                nc.vector.tensor_copy(
                    qk_sb[ftile][:, sh * 512:(sh + 1) * 512], ps[:])

        wv_sb = []
        for et in range(ET):
            t = wvp.tile([128, E], MDT, tag=f"wv{et}", name=f"wv{et}")
            nc.sync.dma_start(out=t[:], in_=wvT_d[et * 128:(et + 1) * 128, :])
            wv_sb.append(t)
        for st in range(ST):
            va3 = v_sb[st][:].rearrange("p (h c) -> p h c", c=65)
            nc.vector.memset(va3[:, :, 64:65], 1.0)
            for fh in range(2):
                ps = psA.tile([128, 512], F32, tag="psA")
                for et in range(ET):
                    nc.tensor.matmul(
                        ps[:],
                        x_sb[et][:, st * 128:(st + 1) * 128],
                        wv_sb[et][:, fh * 512:(fh + 1) * 512],
                        start=(et == 0), stop=(et == ET - 1),
                    )
                # scatter 8 heads' [128,64] blocks into the 65-strided aug layout
                nc.vector.tensor_copy(
                    va3[:, fh * 8:(fh + 1) * 8, 0:64],
                    ps[:].rearrange("p (h c) -> p h c", c=64),
                )
        psA.release()
        wvp.release()
        wsp.release()

        # ---------------- Stage B: attention ----------------
        # Heads run in pairs (2j, 2j+1) sharing Q/K tile j; once the pair's
        # scores are done Q tile j is dead, and the pair's ctx output tile
        # reuses its SBUF slot (same pool tag).
        #
        # Software pipeline across pairs: the scores phase is exp-paced (the
        # scalar engine is the attention bottleneck), so pair j-1's PV
        # accumulation chains are woven between pair j's score groups to fill
        # the PE's exp-wait gaps. The denominator epilogue per (head, half)
        # — DVE reciprocal out of PSUM, ones-matmul broadcast, DVE copy +
        # normalize multiply — is emitted right before the PV chain that
        # reuses its ctx-accumulator PSUM slot, keeping every wait short and
        # the slot recycling deadlock-free.
        atp = tc.alloc_tile_pool(name="attnT", bufs=4 * ST)
        rdp = tc.alloc_tile_pool(name="rden", bufs=4)
        psSC = tc.alloc_tile_pool(name="psSC", bufs=3, space="PSUM")
        psCT = tc.alloc_tile_pool(name="psCT", bufs=2, space="PSUM")
        ctx_sb = [None] * ET
        NP = H // 2
        prev = None  # pipeline state for pair j-1: (pair_at, pcs, rds, rbs)

        def emit_score_group(j, pair_at, hh, kt):
            qt = qk_sb[j]
            kt_t = qk_sb[ET + j]
            off = hh * 64
            ps = psSC.tile([128, S], F32, tag="psSC", name=f"ps{j}_{hh}_{kt}")
            for qh in range(SH):
                # both halves reset their own PSUM bank (start=True); only
                # the last signals group completion — the PE pipelines the
                # pair like an accumulation chain instead of draining between
                # single-shot matmuls.
                nc.tensor.matmul(
                    ps[:, qh * 512:(qh + 1) * 512],
                    kt_t[off:off + 64, kt * 128:(kt + 1) * 128],
                    qt[off:off + 64, qh * 512:(qh + 1) * 512],
                    start=True, stop=(qh == SH - 1),
                    skip_group_check=True,
                )
            nc.scalar.activation(
                pair_at[hh][kt][:], ps[:], AF.Exp,
                bias=shift_ap[:], scale=SCALE)

        def emit_pv_steps(pj, st8, hh, sh, k0, nk):
            # PV accumulation steps k0..k0+nk-1 of chain (hh, sh) for pair pj;
            # allocates the ctx accumulator at step 0 and issues the
            # denominator reciprocal right after the last step.
            pair_at, pcs, rds, rbs = st8
            h = 2 * pj + hh
            if k0 == 0:
                pcs[hh, sh] = psCT.tile([128, 512], F32, tag="psCT",
                                        name=f"pc{pj}_{hh}_{sh}")
            pc = pcs[hh, sh]
            for kt in range(k0, k0 + nk):
                nc.tensor.matmul(
                    pc[0:65, :],
                    v_sb[kt][:, h * 65:h * 65 + 65],
                    pair_at[hh][kt][:, sh * 512:(sh + 1) * 512],
                    start=(kt == 0), stop=(kt == ST - 1),
                )
            if k0 + nk == ST:
                # DVE reciprocal costs ~4us for 512 free elems (measured,
                # cost is free-size-driven) — so it is issued here, two
                # blocks before its consumer (the broadcast matmul), and the
                # ctx rows are copied out unnormalized right after so the
                # PSUM accumulator frees early. The in-place normalize
                # multiply happens later in emit_norm.
                rd = rdp.tile([1, 512], SDT, tag="rden",
                              name=f"rd{pj}_{hh}_{sh}")
                with nc.allow_low_precision(
                        reason="f32r == f32 bits; tag for 1cyc matmul"):
                    nc.vector.reciprocal(rd[:], pc[64:65, :])
                rds[hh, sh] = rd
                off = hh * 64
                nc.vector.tensor_copy(
                    ctx_sb[pj][off:off + 64, sh * 512:(sh + 1) * 512],
                    pc[0:64, :])

        def emit_norm(pj, st8, hh, sh, pool=None):
            # broadcast recip over 64 partitions with a ones matmul (the
            # reciprocal was issued two blocks ago so the PE doesn't wait),
            # then normalize the ctx rows in place — the DVE multiply reads
            # the broadcast straight from PSUM (one PSUM operand is legal).
            # In-slot norms borrow a scores slot; the suffix norms borrow the
            # (by then idle) ctx-accumulator slots so the next slot's score
            # tiles never wait on this slot's trailing DVE multiplies.
            pair_at, pcs, rds, rbs = st8
            off = hh * 64
            if pool is None:
                pb = psSC.tile([128, S], F32, tag="psSC",
                               name=f"pb{pj}_{hh}_{sh}")
            else:
                pb = psCT.tile([128, 512], F32, tag="psCT",
                               name=f"pb{pj}_{hh}_{sh}")
            nc.tensor.matmul(pb[0:64, 0:512], ones_r[:], rds[hh, sh][:],
                             start=True, stop=True)
            ctx = ctx_sb[pj][off:off + 64, sh * 512:(sh + 1) * 512]
            nc.vector.tensor_tensor(ctx, pb[0:64, 0:512], ctx, op=OP.mult)

        for slot in range(NP + 1):
            j = slot        # scores for pair j (if j < NP)
            pj = slot - 1   # PV + epilogue for pair pj (if pj >= 0)
            cur = None
            if j < NP:
                pair_at = []
                for hh in range(2):
                    h = 2 * j + hh
                    pair_at.append(
                        [atp.tile([128, S], BF16, tag="attnT",
                                  name=f"at{h}_{i}") for i in range(ST)])
                cur = (pair_at, {}, {}, {})
            if pj >= 0:
                ctx_sb[pj] = qkp.tile([128, S], ODT, tag=f"qk{pj}",
                                      name=f"ctxT{pj}")
            if cur is not None:
                # 16 score groups alternating heads (hh = g % 2) so adjacent
                # score matmuls sit on different PE row quadrants — the next
                # LDWEIGHTS overlaps the running matmul, which same-quadrant
                # back-to-back matmuls cannot do (measured 509 vs 209 ns).
                # 2 PV steps of pair pj woven after each group (chain g//4,
                # steps 2*(g%4)..+1) keep the PE stream full while exp paces
                # the scores. Norms are placed 2+ groups after their chain's
                # reciprocal, and always before the chain that recycles
                # their PSUM slot.
                # 4 blocks: [4 score groups, head-alternating] + [one full PV
                # chain]. Coarse blocks amortize the PE's dtype/tile-config
                # switch cost (~400ns per switch, measured); psSC bufs=3
                # gives the scores a 3-group exp lookahead so the PE rarely
                # stalls mid-block.
                for blk in range(4):
                    if prev is not None and blk >= 2:
                        emit_norm(pj, prev, 0, blk - 2)
                    for kt in (2 * blk, 2 * blk + 1):
                        emit_score_group(j, pair_at, 0, kt)
                        emit_score_group(j, pair_at, 1, kt)
                    if prev is not None:
                        emit_pv_steps(pj, prev, blk // 2, blk % 2, 0, ST)
                if prev is not None:
                    emit_norm(pj, prev, 1, 0, pool=psCT)
                    emit_norm(pj, prev, 1, 1, pool=psCT)
            else:
                # drain slot: last pair's PV + epilogue, no scores to weave
                emit_pv_steps(pj, prev, 0, 0, 0, ST)
                emit_pv_steps(pj, prev, 0, 1, 0, ST)
                emit_norm(pj, prev, 0, 0)
                emit_pv_steps(pj, prev, 1, 0, 0, ST)
                emit_norm(pj, prev, 0, 1)
                emit_pv_steps(pj, prev, 1, 1, 0, ST)
                emit_norm(pj, prev, 1, 0)
                emit_norm(pj, prev, 1, 1)
            prev = cur
        psCT.release()
        psSC.release()
        rdp.release()
        atp.release()
        vap.release()

        # ---- LayerNorm helpers: sum-matmuls interleaved into the producer's
        # PE stream (one et deferred so the PE never waits on the residual),
        # then a short stats chain and per-et normalize.
        def ln_mms(et, src_tiles, sq_tiles, ps_sum, ps_sq, lnp):
            sq = lnp.tile([128, S], MDT, tag="lnsq", bufs=2, name=f"sq{et}")
            nc.scalar.activation(sq[:], src_tiles[et][:], AF.Square)
            sq_tiles[et] = sq
            for sh in range(SH):
                sl = slice(sh * 512, (sh + 1) * 512)
                nc.tensor.matmul(
                    ps_sum[:, sl], ones128[:], src_tiles[et][:, sl],
                    start=(et == 0), stop=(et == ET - 1))
                nc.tensor.matmul(
                    ps_sq[:, sl], ones128[:], sq[:, sl],
                    start=(et == 0), stop=(et == ET - 1))

        def ln_finalize(src_tiles, dst_tiles, g_ap, b_ap, ps_sum, ps_sq, lnp,
                        per_et_done=None):
            # ps_sum == mean, ps_sq == E[x^2] (ones128 holds 1/E).
            mu = lnp.tile([128, S], F32, tag="lnmu")
            nc.vector.tensor_copy(mu[:], ps_sum[:])
            musq = lnp.tile([128, S], F32, tag="lnmusq")
            nc.vector.tensor_tensor(musq[:], mu[:], mu[:], op=OP.mult)
            var = lnp.tile([128, S], F32, tag="lnvar")
            nc.vector.tensor_tensor(var[:], ps_sq[:], musq[:], op=OP.subtract)
            std = lnp.tile([128, S], F32, tag="lnstd")
            nc.scalar.activation(std[:], var[:], AF.Sqrt, bias=eps_ap[:])
            rstd = lnp.tile([128, S], F32, tag="lnrstd")
            nc.vector.reciprocal(rstd[:], std[:])
            for et in range(ET):
                t1 = lnp.tile([128, S], F32, tag="lnt1", bufs=2, name=f"t1{et}")
                nc.vector.tensor_tensor(t1[:], src_tiles[et][:], mu[:],
                                        op=OP.subtract)
                nc.vector.tensor_tensor(t1[:], t1[:], rstd[:], op=OP.mult)
                nc.scalar.activation(
                    dst_tiles[et][:], t1[:], AF.Identity,
                    bias=b_ap[:, et:et + 1], scale=g_ap[:, et:et + 1])
                if per_et_done is not None:
                    per_et_done(et)

        # -------- Stage C: out-proj + residual (in place in x) + LN1 --------
        wo_r = woT_d.rearrange("(a p) f -> p a f", p=128)
        ln1p = tc.alloc_tile_pool(name="ln1", bufs=1)
        psLN1 = tc.alloc_tile_pool(name="psLN1", bufs=1, space="PSUM")
        wcp = tc.alloc_tile_pool(name="wslabC", bufs=3)
        psC = tc.alloc_tile_pool(name="psC", bufs=3, space="PSUM")
        ps1_sum = psLN1.tile([128, S], F32, tag="psLNsum")
        ps1_sq = psLN1.tile([128, S], F32, tag="psLNsq")
        sq1_tiles = [None] * ET
        for et in range(ET):
            slab = wcp.tile([128, ET * 128], ODT, tag="wslabC")
            nc.sync.dma_start(
                out=slab[:].rearrange("p (a f) -> p a f", a=ET),
                in_=wo_r[:, :, et * 128:(et + 1) * 128])
            for sh in range(SH):
                sl = slice(sh * 512, (sh + 1) * 512)
                ps = psC.tile([128, 512], F32, tag="psC")
                for kt in range(ET):
                    nc.tensor.matmul(
                        ps[:], slab[:, kt * 128:(kt + 1) * 128],
                        ctx_sb[kt][:, sl],
                        start=(kt == 0), stop=(kt == ET - 1))
                # residual in place: x tile becomes hpre
                nc.vector.tensor_tensor(
                    x_sb[et][:, sl], ps[:], x_sb[et][:, sl], op=OP.add)
            if et > 0:
                ln_mms(et - 1, x_sb, sq1_tiles, ps1_sum, ps1_sq, ln1p)
        ln_mms(ET - 1, x_sb, sq1_tiles, ps1_sum, ps1_sq, ln1p)
        psC.release()
        wcp.release()

        # LN1 in place: x tiles (hpre) -> h
        ln_finalize(x_sb, x_sb, g1s, be1s, ps1_sum, ps1_sq, ln1p)
        hT_sb = x_sb  # x tiles now hold h
        psLN1.release()
        ln1p.release()
        qkp.release()

        # ---------------- Stage D: FFN + residual + LN2 ----------------
        w1_r = w1T_d.rearrange("(a p) f -> p a f", p=128)
        w2_r = w2T_d.rearrange("(a p) e -> p a e", p=128)
        ln2p = tc.alloc_tile_pool(name="ln2", bufs=1)
        psLN2 = tc.alloc_tile_pool(name="psLN2", bufs=1, space="PSUM")
        ps2_sum = psLN2.tile([128, S], F32, tag="psLNsum")
        ps2_sq = psLN2.tile([128, S], F32, tag="psLNsq")
        psD = tc.alloc_tile_pool(name="psD", bufs=3, space="PSUM")
        zp = tc.alloc_tile_pool(name="z", bufs=1)
        z_sb = [zp.tile([128, S], BF16, tag=f"z{ft}", name=f"z{ft}")
                for ft in range(FT1)]
        w2p = tc.alloc_tile_pool(name="w2slab", bufs=2)
        wdp = tc.alloc_tile_pool(name="wslabD", bufs=3)
        for ft in range(FT1):
            slab = wdp.tile([128, ET * 128], MDT, tag="wslabD")
            nc.sync.dma_start(
                out=slab[:].rearrange("p (a f) -> p a f", a=ET),
                in_=w1_r[:, :, ft * 128:(ft + 1) * 128])
            for sh in range(SH):
                sl = slice(sh * 512, (sh + 1) * 512)
                ps = psD.tile([128, 512], F32, tag="psD")
                for et in range(ET):
                    nc.tensor.matmul(
                        ps[:],
                        slab[:, et * 128:(et + 1) * 128],
                        hT_sb[et][:, sl],
                        start=(et == 0), stop=(et == ET - 1))
                nc.scalar.activation(
                    z_sb[ft][:, sl], ps[:], AF.Relu,
                    bias=b1s[:, ft:ft + 1])
        wdp.release()

        # prefetch the first w2 slabs while FFN1 drains
        w2slabs = [None, None]
        for et in range(2):
            w2slab_t = w2p.tile([128, FT1 * 128], BF16, tag="w2slab",
                                name=f"w2slab{et}")
            w2slabs[et] = w2slab_t
            nc.sync.dma_start(
                out=w2slab_t[:].rearrange("p (a e) -> p a e", a=FT1),
                in_=w2_r[:, :, et * 128:(et + 1) * 128])

        sq2_tiles = [None] * ET
        for et in range(ET):
            w2slab = w2slabs[et % 2]
            for sh in range(SH):
                sl = slice(sh * 512, (sh + 1) * 512)
                ps = psD.tile([128, 512], F32, tag="psD")
                for ftk in range(FT1):
                    nc.tensor.matmul(
                        ps[:],
                        w2slab[:, ftk * 128:(ftk + 1) * 128],
                        z_sb[ftk][:, sl],
                        start=(ftk == 0), stop=(ftk == FT1 - 1))
                # y = ffn2 + b2 + h, in place: x tile becomes y
                nc.vector.scalar_tensor_tensor(
                    x_sb[et][:, sl], ps[:], b2s[:, et:et + 1],
                    hT_sb[et][:, sl], op0=OP.add, op1=OP.add)
            if et + 2 < ET:
                w2slab_t = w2p.tile([128, FT1 * 128], BF16, tag="w2slab",
                                    name=f"w2slab{et + 2}")
                w2slabs[et % 2] = w2slab_t
                nc.sync.dma_start(
                    out=w2slab_t[:].rearrange("p (a e) -> p a e", a=FT1),
                    in_=w2_r[:, :, (et + 2) * 128:(et + 3) * 128])
            if et > 0:
                ln_mms(et - 1, x_sb, sq2_tiles, ps2_sum, ps2_sq, ln2p)
        ln_mms(ET - 1, x_sb, sq2_tiles, ps2_sum, ps2_sq, ln2p)
        w2p.release()
        zp.release()
        psD.release()

        # LN2 in place: x tiles (y) -> out, DMA out per et as it lands
        def dma_out(et):
            nc.sync.dma_start(
                out=out_d[et * 128:(et + 1) * 128, :], in_=x_sb[et][:])

        ln_finalize(x_sb, x_sb, g2s, be2s, ps2_sum, ps2_sq, ln2p,
                    per_et_done=dma_out)
        psLN2.release()
        ln2p.release()
        xp.release()
        cpool.release()
    _split_multi_waits(nc)
    return nc


def prep_inputs(x, in_proj_w, out_proj_w, ln1_g, ln1_b, ln2_g, ln2_b,
                w1, b1, w2, b2):
    """Host-side reshapes/transposes. Returns (shared weight map, per-core xT)."""
    f32 = np.float32
    ET = E // 128

    def pcols(v, n):  # [n*128] vector -> [128, n] per-partition column layout
        return np.ascontiguousarray(np.asarray(v, f32).reshape(n, 128).T)

    shared = {
        "ones128": np.full((128, 128), 1.0 / E, f32),
        "onesr": np.ones((1, 64), f32),
        "wqkT": np.ascontiguousarray(np.asarray(in_proj_w, f32)[:2 * E].T),
        "wvT": np.ascontiguousarray(np.asarray(in_proj_w, f32)[2 * E:].T),
        "woT": np.ascontiguousarray(np.asarray(out_proj_w, f32).T),
        "w1T": np.ascontiguousarray(np.asarray(w1, f32).T),
        "w2T": np.ascontiguousarray(np.asarray(w2, f32).T).astype(
            ml_dtypes.bfloat16),
        "b1t": pcols(b1, FF // 128),
        "b2t": pcols(b2, ET),
        "g1t": pcols(ln1_g, ET),
        "be1t": pcols(ln1_b, ET),
        "g2t": pcols(ln2_g, ET),
        "be2t": pcols(ln2_b, ET),
    }
    x = np.asarray(x, f32)
    xTs = [np.ascontiguousarray(x[b].T) for b in range(x.shape[0])]
    return shared, xTs


def kernel(x, in_proj_w, out_proj_w, ln1_g, ln1_b, ln2_g, ln2_b,
           w1, b1, w2, b2, _trace=False, _cfg=None):
    S = x.shape[1]
    nc = build_bass(S=S, cfg=_cfg)
    shared, xTs = prep_inputs(x, in_proj_w, out_proj_w, ln1_g, ln1_b,
                              ln2_g, ln2_b, w1, b1, w2, b2)
    in_maps = [dict(shared, xT=xTs[b]) for b in range(x.shape[0])]
    res = run_bass_kernel_spmd(nc, in_maps, core_ids=list(range(NCORES)),
                               trace=_trace)
    out = np.stack([np.asarray(res.results[b]["outT"], np.float32).T
                    for b in range(x.shape[0])])
    if _trace:
        kernel.last_exec_time_ns = res.exec_time_ns
        kernel.last_results = res
    return out


# revision 38
# speedup vs baseline: 1.0419x; 1.0419x over previous
"""Trainium2 Bass kernel for nn_AttentionBlock (B=8,S=1024,E=1024,H=16,FF=4096).

Strategy: pure data-parallel over batch — each of the 8 NeuronCores runs the
full attention block on one [S,E] slice. No collectives.

Per-core layout convention: every activation lives feature-major ("T" =
[feature, token]) in SBUF so that each matmul consumes the previous output
directly (weights are pre-transposed on the host; the TensorEngine computes
lhsT.T @ rhs). All f32 matmul operands are stored as float32r, which the PE
runs single-pass (fp32_mode=HIGH, ~1 cyc/row at 512-wide moving dim) instead
of the 4-pass full-precision fp32 mode. Softmax uses a constant logit shift
(no max pass — logits are bounded well inside fp32 exp range for this scale),
the denominator comes from a ones-column appended to V, and normalization is
applied to the [64, S] context rows rather than the [S, S] score matrix.

Attention epilogue keeps the PE stream free of round-trips: the reciprocal
runs on DVE straight out of PSUM, the across-partition broadcast runs on the
(otherwise idle) GPSIMD engine, and the normalize multiply on DVE — no PE
broadcast matmuls, so PSUM fits scores (2x[128,S]) + 4 ctx accumulators.

LayerNorm reduces over the partition axis via (1/E)-scaled all-ones matmuls
whose [128, S] PSUM output doubles as the partition-broadcast of the per-token
mean / second moment; those matmuls are interleaved one-et-deferred into the
producing GEMM's PE stream so the PE never waits on the residual adds. The
gamma/beta application rides the scalar engine (Identity activation with
per-partition scale/bias).

SBUF slot reuse (pool release is LIFO, so lifetimes must nest): the ctx tiles
take over the dead Q tiles' slots (heads run in pairs so Q tile j is fully
dead first), and residual/LN/FFN epilogues run in place in the x tiles, which
successively hold x -> hpre -> h -> y -> out.
"""
import math
import numpy as np
import ml_dtypes

import concourse.bass as bass
import concourse.mybir as mybir
from concourse.tile import TileContext
from concourse.bass_utils import run_bass_kernel_spmd
from concourse.vector_clock import ScopedClock, VectorClock


def _split_drain_and_barrier(self, tick_clock, wait_clock):
    """Replacement for TileContext._drain_and_barrier: this walrus build
    allows only ONE sync-wait command on NoOp/Drain instructions, so the
    end-of-kernel drain's per-processor waits are split across single-wait
    SP nops (the SP sequencer is in-order, so by the drain every condition
    holds)."""
    gc = tick_clock.global_clock
    n = len(gc)
    for i in range(n):
        if gc[i] <= 0:
            continue
        vc = VectorClock([gc[j] if j == i else 0 for j in range(n)])
        nop_inst = self.nc.sync.nop()
        wait_clock.add_sem_waits(nop_inst.ins, ScopedClock({None: vc}))
    self.nc.sync.drain()
    self.nc.all_engine_barrier()
    assert self.sems is not None
    popped = self.nc._tile_sem_poison_stack.pop()
    assert popped is self._sem_poison
    self.nc.clear_and_free_semaphores(list(self.sems.allocated().values()))
    self.nc.all_engine_barrier()


TileContext._drain_and_barrier = _split_drain_and_barrier


def _split_multi_waits(nc):
    """This walrus build supports a single sync-wait command per instruction.
    Hoist all but one wait of any instruction onto fresh single-wait NoOps on
    the same engine, inserted immediately before it (engine queues are
    in-order, so the semantics are identical)."""
    ctr = 0

    def walk(blocks):
        nonlocal ctr
        for b in blocks:
            il = b.instructions
            i = 0
            while i < len(il):
                inst = il[i]
                si = inst.sync_info
                waits = list(si.on_wait) if (si is not None and si.on_wait) else []
                if len(waits) > 1:
                    for w in waits[:-1]:
                        ctr += 1
                        nop = mybir.InstNoOp(
                            name=f"I-wsplit-{ctr}", engine=inst.engine,
                            ins=[], outs=[])
                        nop.sync_info = mybir.SyncInfo(on_wait=[w], on_update=[])
                        nc.register_instruction(nop, overwrite=True)
                        il.insert(i, nop)
                        i += 1
                    inst.sync_info = mybir.SyncInfo(
                        on_wait=[waits[-1]],
                        on_update=list(si.on_update) if si.on_update else [])
                i += 1
            sub = getattr(b, "blocks", None)
            if sub:
                walk(sub)

    for f in nc.m.functions:
        walk(f.blocks)

F32 = mybir.dt.float32
F32R = mybir.dt.float32r
BF16 = mybir.dt.bfloat16
AF = mybir.ActivationFunctionType
OP = mybir.AluOpType

B, E, H, FF = 8, 1024, 16, 4096
HD = E // H  # 64
N_DOM = 1024
SCALE = math.sqrt(1.0 / HD) * 2.0 * math.log(N_DOM)  # 1.73287
SHIFT = -40.0  # constant logit shift inside exp; see module docstring
LN_EPS = 1e-5
NCORES = 8

# Per-matmul-group compute dtype for f32-stored operands: F32R (single-pass
# "HIGH" mode, ~1 cyc/row at 512-wide moving dim, bf16-grade multiply
# precision with f32 accumulate) or F32 (4-pass, exact, 4x slower).
DEFAULT_CFG = {
    "main": F32R,
    "scores": F32R,
    "outp": F32R,
}


def build_bass(S=1024, cfg=None):
    cfg = dict(DEFAULT_CFG, **(cfg or {}))
    MDT = cfg["main"]      # dtype of x/h/y tiles, qkv+ffn1 weights, LN ones
    SDT = cfg["scores"]    # dtype of Q/K tiles
    ODT = cfg["outp"]      # dtype of ctx tiles + out-proj weights
    ET = E // 128          # 8 e-tiles
    ST = S // 128          # s-tiles
    SH = S // 512          # 512-wide column halves
    FT1 = FF // 128        # 32 f-tiles for FFN hidden

    nc = bass.Bass()
    xT_d = nc.declare_dram_parameter("xT", [E, S], MDT, isOutput=False)
    wqkT_d = nc.declare_dram_parameter("wqkT", [E, 2 * E], MDT, isOutput=False)
    wvT_d = nc.declare_dram_parameter("wvT", [E, E], MDT, isOutput=False)
    woT_d = nc.declare_dram_parameter("woT", [E, E], ODT, isOutput=False)
    w1T_d = nc.declare_dram_parameter("w1T", [E, FF], MDT, isOutput=False)
    w2T_d = nc.declare_dram_parameter("w2T", [FF, E], BF16, isOutput=False)
    b1_d = nc.declare_dram_parameter("b1t", [128, FF // 128], F32, isOutput=False)
    b2_d = nc.declare_dram_parameter("b2t", [128, ET], F32, isOutput=False)
    g1_d = nc.declare_dram_parameter("g1t", [128, ET], F32, isOutput=False)
    be1_d = nc.declare_dram_parameter("be1t", [128, ET], F32, isOutput=False)
    g2_d = nc.declare_dram_parameter("g2t", [128, ET], F32, isOutput=False)
    be2_d = nc.declare_dram_parameter("be2t", [128, ET], F32, isOutput=False)
    ones_d = nc.declare_dram_parameter("ones128", [128, 128], MDT, isOutput=False)
    onesr_d = nc.declare_dram_parameter("onesr", [1, 64], SDT, isOutput=False)
    out_d = nc.declare_dram_parameter("outT", [E, S], MDT, isOutput=True)

    with TileContext(nc) as tc:
        cpool = tc.alloc_tile_pool(name="consts", bufs=1)
        xp = tc.alloc_tile_pool(name="xp", bufs=1)

        # ones128 holds 1/E so the LN sum-matmuls directly produce the
        # partition-broadcast mean / second moment in PSUM.
        ones128 = cpool.tile([128, 128], MDT, tag="ones128")
        nc.sync.dma_start(out=ones128[:], in_=ones_d[:])
        ones_r = cpool.tile([1, 64], SDT, tag="ones_r")
        nc.sync.dma_start(out=ones_r[:], in_=onesr_d[:])
        shift_ap = cpool.tile([128, 1], F32, tag="shift")
        nc.vector.memset(shift_ap[:], SHIFT)
        eps_ap = cpool.tile([128, 1], F32, tag="eps")
        nc.vector.memset(eps_ap[:], LN_EPS)
        b1s = cpool.tile([128, FF // 128], F32, tag="b1s")
        nc.sync.dma_start(out=b1s[:], in_=b1_d[:])
        b2s = cpool.tile([128, ET], F32, tag="b2s")
        nc.sync.dma_start(out=b2s[:], in_=b2_d[:])
        g1s = cpool.tile([128, ET], F32, tag="g1s")
        nc.sync.dma_start(out=g1s[:], in_=g1_d[:])
        be1s = cpool.tile([128, ET], F32, tag="be1s")
        nc.sync.dma_start(out=be1s[:], in_=be1_d[:])
        g2s = cpool.tile([128, ET], F32, tag="g2s")
        nc.sync.dma_start(out=g2s[:], in_=g2_d[:])
        be2s = cpool.tile([128, ET], F32, tag="be2s")
        nc.sync.dma_start(out=be2s[:], in_=be2_d[:])

        x_sb = []
        for et in range(ET):
            t = xp.tile([128, S], MDT, tag=f"x{et}", name=f"x{et}")
            nc.sync.dma_start(out=t[:], in_=xT_d[et * 128:(et + 1) * 128, :])
            x_sb.append(t)

        # ---------------- Stage A: QKV projection ----------------
        qkp = tc.alloc_tile_pool(name="qk", bufs=1)
        vap = tc.alloc_tile_pool(name="va", bufs=1)
        qk_sb = [qkp.tile([128, S], SDT, tag=f"qk{j}", name=f"qk{j}")
                 for j in range(2 * ET)]
        v_sb = [vap.tile([128, 16 * 65], BF16, tag=f"va{st}", name=f"va{st}")
                for st in range(ST)]
        wqk_r = wqkT_d.rearrange("(a p) f -> p a f", p=128)
        wsp = tc.alloc_tile_pool(name="wslabA", bufs=3)
        wvp = tc.alloc_tile_pool(name="wv", bufs=1)
        psA = tc.alloc_tile_pool(name="psA", bufs=4, space="PSUM")
        for ftile in range(2 * ET):
            slab = wsp.tile([128, ET * 128], MDT, tag="wslabA")
            nc.sync.dma_start(
                out=slab[:].rearrange("p (a f) -> p a f", a=ET),
                in_=wqk_r[:, :, ftile * 128:(ftile + 1) * 128],
            )
            for sh in range(SH):
                ps = psA.tile([128, 512], F32, tag="psA")
                for et in range(ET):
                    nc.tensor.matmul(
                        ps[:],
                        slab[:, et * 128:(et + 1) * 128],
                        x_sb[et][:, sh * 512:(sh + 1) * 512],
                        start=(et == 0), stop=(et == ET - 1),
                    )
                nc.vector.tensor_copy(
                    qk_sb[ftile][:, sh * 512:(sh + 1) * 512], ps[:])

        wv_sb = []
        for et in range(ET):
            t = wvp.tile([128, E], MDT, tag=f"wv{et}", name=f"wv{et}")
            nc.sync.dma_start(out=t[:], in_=wvT_d[et * 128:(et + 1) * 128, :])
            wv_sb.append(t)
        for st in range(ST):
            va3 = v_sb[st][:].rearrange("p (h c) -> p h c", c=65)
            nc.vector.memset(va3[:, :, 64:65], 1.0)
            for fh in range(2):
                ps = psA.tile([128, 512], F32, tag="psA")
                for et in range(ET):
                    nc.tensor.matmul(
                        ps[:],
                        x_sb[et][:, st * 128:(st + 1) * 128],
                        wv_sb[et][:, fh * 512:(fh + 1) * 512],
                        start=(et == 0), stop=(et == ET - 1),
                    )
                # scatter 8 heads' [128,64] blocks into the 65-strided aug layout
                nc.vector.tensor_copy(
                    va3[:, fh * 8:(fh + 1) * 8, 0:64],
                    ps[:].rearrange("p (h c) -> p h c", c=64),
                )
        psA.release()
        wvp.release()
        wsp.release()

        # ---------------- Stage B: attention ----------------
        # Heads run in pairs (2j, 2j+1) sharing Q/K tile j; once the pair's
        # scores are done Q tile j is dead, and the pair's ctx output tile
        # reuses its SBUF slot (same pool tag).
        #
        # Three-deep software pipeline across pairs: slot j runs pair j's
        # exp-paced scores, weaves pair j-1's PV accumulation chains between
        # the score blocks (coarse blocks amortize the PE's measured ~400ns
        # dtype/tile-config switch cost), and applies pair j-2's deferred
        # softmax normalization. Each PV chain ends by copying its ctx rows
        # out unnormalized and its denominator row to SBUF, so the PSUM
        # accumulator frees within ~1.5us. The four denominator rows take a
        # DMA round trip through DRAM that transposes them into a [128,16]
        # tile: ONE partition-parallel DVE reciprocal covers the whole pair
        # (a [1,512] reciprocal costs 4us — free-size-driven — which
        # previously made DVE the attention bottleneck), then DMA transposes
        # back to rows for the ones-matmul partition broadcast and in-place
        # normalize multiply one slot later.
        atp = tc.alloc_tile_pool(name="attnT", bufs=4 * ST)
        rdp = tc.alloc_tile_pool(name="rden", bufs=4)
        psSC = tc.alloc_tile_pool(name="psSC", bufs=3, space="PSUM")
        psCT = tc.alloc_tile_pool(name="psCT", bufs=2, space="PSUM")
        NP = H // 2
        scrA = nc.dram_tensor("denscrA", [NP, 4, 512], SDT, kind="Internal")
        scrB = nc.dram_tensor("denscrB", [NP, 4, 512], SDT, kind="Internal")
        ctx_sb = [None] * ET

        def emit_score_group(j, pair_at, hh, kt):
            qt = qk_sb[j]
            kt_t = qk_sb[ET + j]
            off = hh * 64
            ps = psSC.tile([128, S], F32, tag="psSC", name=f"ps{j}_{hh}_{kt}")
            for qh in range(SH):
                nc.tensor.matmul(
                    ps[:, qh * 512:(qh + 1) * 512],
                    kt_t[off:off + 64, kt * 128:(kt + 1) * 128],
                    qt[off:off + 64, qh * 512:(qh + 1) * 512],
                    start=True, stop=(qh == SH - 1),
                    skip_group_check=True,
                )
            nc.scalar.activation(
                pair_at[hh][kt][:], ps[:], AF.Exp,
                bias=shift_ap[:], scale=SCALE)

        def emit_pv_chain(pj, st8, hh, sh):
            # 8 PV accumulation steps, then: denominator row -> SBUF -> DRAM
            # (for the batched reciprocal), ctx rows -> SBUF unnormalized.
            # Both copies free the PSUM accumulator ~1.5us after the chain.
            pair_at = st8["at"]
            h = 2 * pj + hh
            c = 2 * hh + sh
            pc = psCT.tile([128, 512], F32, tag="psCT", name=f"pc{pj}_{hh}_{sh}")
            for kt in range(ST):
                nc.tensor.matmul(
                    pc[0:65, :],
                    v_sb[kt][:, h * 65:h * 65 + 65],
                    pair_at[hh][kt][:, sh * 512:(sh + 1) * 512],
                    start=(kt == 0), stop=(kt == ST - 1),
                )
            denrow = rdp.tile([1, 512], SDT, tag="denrow", bufs=4,
                              name=f"dr{pj}_{c}")
            with nc.allow_low_precision(reason="f32r == f32 bits"):
                nc.vector.tensor_copy(denrow[:], pc[64:65, :])
            nc.sync.dma_start(out=scrA[pj, c, :], in_=denrow[:])
            off = hh * 64
            nc.vector.tensor_copy(
                ctx_sb[pj][off:off + 64, sh * 512:(sh + 1) * 512],
                pc[0:64, :])

        def emit_den_recip(pj, st8):
            # gather the pair's 4 denominator rows transposed into [128,16],
            # one partition-parallel reciprocal, scatter back to rows.
            dent = rdp.tile([128, 16], SDT, tag="dent", bufs=2,
                            name=f"dent{pj}")
            nc.sync.dma_start(
                out=dent[:],
                in_=scrA[pj].rearrange("c (f p) -> p (c f)", p=128))
            dentr = rdp.tile([128, 16], SDT, tag="dentr", bufs=2,
                             name=f"dentr{pj}")
            with nc.allow_low_precision(reason="f32r == f32 bits"):
                nc.vector.reciprocal(dentr[:], dent[:])
            nc.sync.dma_start(
                out=scrB[pj].rearrange("c (f p) -> p (c f)", p=128),
                in_=dentr[:])
            for c in range(4):
                rdrow = rdp.tile([1, 512], SDT, tag="rdrow", bufs=8,
                                 name=f"rr{pj}_{c}")
                nc.sync.dma_start(out=rdrow[:], in_=scrB[pj, c, :])
                st8["rd"][c] = rdrow

        def emit_norm(qj, st8, hh, sh, pool=None):
            # ones-matmul broadcast of the reciprocal row (ready since last
            # slot), then in-place normalize; the DVE multiply reads the
            # broadcast straight from PSUM (one PSUM operand is legal).
            pb = (pool or psSC).tile(
                [128, 512] if pool is not None else [128, S],
                F32, tag="psCT" if pool is not None else "psSC",
                name=f"pb{qj}_{hh}_{sh}")
            nc.tensor.matmul(pb[0:64, 0:512], ones_r[:],
                             st8["rd"][2 * hh + sh][:], start=True, stop=True)
            off = hh * 64
            ctx = ctx_sb[qj][off:off + 64, sh * 512:(sh + 1) * 512]
            nc.vector.tensor_tensor(ctx, pb[0:64, 0:512], ctx, op=OP.mult)

        states = {}
        for slot in range(NP + 2):
            j = slot if slot < NP else None
            pj = slot - 1 if 0 <= slot - 1 < NP else None
            qj = slot - 2 if slot - 2 >= 0 else None
            if j is not None:
                pair_at = []
                for hh in range(2):
                    h = 2 * j + hh
                    pair_at.append(
                        [atp.tile([128, S], BF16, tag="attnT",
                                  name=f"at{h}_{i}") for i in range(ST)])
                states[j] = {"at": pair_at, "rd": {}}
            if pj is not None:
                ctx_sb[pj] = qkp.tile([128, S], ODT, tag=f"qk{pj}",
                                      name=f"ctxT{pj}")
            for blk in range(4):
                if qj is not None and blk >= 1:
                    emit_norm(qj, states[qj], (blk - 1) // 2, (blk - 1) % 2)
                if j is not None:
                    for kt in (2 * blk, 2 * blk + 1):
                        emit_score_group(j, pair_at, 0, kt)
                        emit_score_group(j, pair_at, 1, kt)
                if pj is not None:
                    emit_pv_chain(pj, states[pj], blk // 2, blk % 2)
            if qj is not None:
                emit_norm(qj, states[qj], 1, 1,
                          pool=psCT if j is not None else None)
                del states[qj]
            if pj is not None:
                emit_den_recip(pj, states[pj])
        psCT.release()
        psSC.release()
        rdp.release()
        atp.release()
        vap.release()

        # ---- LayerNorm helpers: sum-matmuls interleaved into the producer's
        # PE stream (one et deferred so the PE never waits on the residual),
        # then a short stats chain and per-et normalize.
        def ln_mms(et, src_tiles, sq_tiles, ps_sum, ps_sq, lnp):
            sq = lnp.tile([128, S], MDT, tag="lnsq", bufs=2, name=f"sq{et}")
            nc.scalar.activation(sq[:], src_tiles[et][:], AF.Square)
            sq_tiles[et] = sq
            for sh in range(SH):
                sl = slice(sh * 512, (sh + 1) * 512)
                nc.tensor.matmul(
                    ps_sum[:, sl], ones128[:], src_tiles[et][:, sl],
                    start=(et == 0), stop=(et == ET - 1))
                nc.tensor.matmul(
                    ps_sq[:, sl], ones128[:], sq[:, sl],
                    start=(et == 0), stop=(et == ET - 1))

        def ln_finalize(src_tiles, dst_tiles, g_ap, b_ap, ps_sum, ps_sq, lnp,
                        per_et_done=None):
            # ps_sum == mean, ps_sq == E[x^2] (ones128 holds 1/E).
            mu = lnp.tile([128, S], F32, tag="lnmu")
            nc.vector.tensor_copy(mu[:], ps_sum[:])
            musq = lnp.tile([128, S], F32, tag="lnmusq")
            nc.vector.tensor_tensor(musq[:], mu[:], mu[:], op=OP.mult)
            var = lnp.tile([128, S], F32, tag="lnvar")
            nc.vector.tensor_tensor(var[:], ps_sq[:], musq[:], op=OP.subtract)
            std = lnp.tile([128, S], F32, tag="lnstd")
            nc.scalar.activation(std[:], var[:], AF.Sqrt, bias=eps_ap[:])
            rstd = lnp.tile([128, S], F32, tag="lnrstd")
            nc.vector.reciprocal(rstd[:], std[:])
            for et in range(ET):
                t1 = lnp.tile([128, S], F32, tag="lnt1", bufs=2, name=f"t1{et}")
                nc.vector.tensor_tensor(t1[:], src_tiles[et][:], mu[:],
                                        op=OP.subtract)
                nc.vector.tensor_tensor(t1[:], t1[:], rstd[:], op=OP.mult)
                nc.scalar.activation(
                    dst_tiles[et][:], t1[:], AF.Identity,
                    bias=b_ap[:, et:et + 1], scale=g_ap[:, et:et + 1])
                if per_et_done is not None:
                    per_et_done(et)

        # -------- Stage C: out-proj + residual (in place in x) + LN1 --------
        wo_r = woT_d.rearrange("(a p) f -> p a f", p=128)
        ln1p = tc.alloc_tile_pool(name="ln1", bufs=1)
        psLN1 = tc.alloc_tile_pool(name="psLN1", bufs=1, space="PSUM")
        wcp = tc.alloc_tile_pool(name="wslabC", bufs=3)
        psC = tc.alloc_tile_pool(name="psC", bufs=3, space="PSUM")
        ps1_sum = psLN1.tile([128, S], F32, tag="psLNsum")
        ps1_sq = psLN1.tile([128, S], F32, tag="psLNsq")
        sq1_tiles = [None] * ET
        for et in range(ET):
            slab = wcp.tile([128, ET * 128], ODT, tag="wslabC")
            nc.sync.dma_start(
                out=slab[:].rearrange("p (a f) -> p a f", a=ET),
                in_=wo_r[:, :, et * 128:(et + 1) * 128])
            for sh in range(SH):
                sl = slice(sh * 512, (sh + 1) * 512)
                ps = psC.tile([128, 512], F32, tag="psC")
                for kt in range(ET):
                    nc.tensor.matmul(
                        ps[:], slab[:, kt * 128:(kt + 1) * 128],
                        ctx_sb[kt][:, sl],
                        start=(kt == 0), stop=(kt == ET - 1))
                # residual in place: x tile becomes hpre
                nc.vector.tensor_tensor(
                    x_sb[et][:, sl], ps[:], x_sb[et][:, sl], op=OP.add)
            if et > 0:
                ln_mms(et - 1, x_sb, sq1_tiles, ps1_sum, ps1_sq, ln1p)
        ln_mms(ET - 1, x_sb, sq1_tiles, ps1_sum, ps1_sq, ln1p)
        psC.release()
        wcp.release()

        # LN1 in place: x tiles (hpre) -> h
        ln_finalize(x_sb, x_sb, g1s, be1s, ps1_sum, ps1_sq, ln1p)
        hT_sb = x_sb  # x tiles now hold h
        psLN1.release()
        ln1p.release()
        qkp.release()

        # ---------------- Stage D: FFN + residual + LN2 ----------------
        w1_r = w1T_d.rearrange("(a p) f -> p a f", p=128)
        w2_r = w2T_d.rearrange("(a p) e -> p a e", p=128)
        ln2p = tc.alloc_tile_pool(name="ln2", bufs=1)
        psLN2 = tc.alloc_tile_pool(name="psLN2", bufs=1, space="PSUM")
        ps2_sum = psLN2.tile([128, S], F32, tag="psLNsum")
        ps2_sq = psLN2.tile([128, S], F32, tag="psLNsq")
        psD = tc.alloc_tile_pool(name="psD", bufs=3, space="PSUM")
        zp = tc.alloc_tile_pool(name="z", bufs=1)
        z_sb = [zp.tile([128, S], BF16, tag=f"z{ft}", name=f"z{ft}")
                for ft in range(FT1)]
        w2p = tc.alloc_tile_pool(name="w2slab", bufs=2)
        wdp = tc.alloc_tile_pool(name="wslabD", bufs=3)
        for ft in range(FT1):
            slab = wdp.tile([128, ET * 128], MDT, tag="wslabD")
            nc.sync.dma_start(
                out=slab[:].rearrange("p (a f) -> p a f", a=ET),
                in_=w1_r[:, :, ft * 128:(ft + 1) * 128])
            for sh in range(SH):
                sl = slice(sh * 512, (sh + 1) * 512)
                ps = psD.tile([128, 512], F32, tag="psD")
                for et in range(ET):
                    nc.tensor.matmul(
                        ps[:],
                        slab[:, et * 128:(et + 1) * 128],
                        hT_sb[et][:, sl],
                        start=(et == 0), stop=(et == ET - 1))
                nc.scalar.activation(
                    z_sb[ft][:, sl], ps[:], AF.Relu,
                    bias=b1s[:, ft:ft + 1])
        wdp.release()

        # prefetch the first w2 slabs while FFN1 drains
        w2slabs = [None, None]
        for et in range(2):
            w2slab_t = w2p.tile([128, FT1 * 128], BF16, tag="w2slab",
                                name=f"w2slab{et}")
            w2slabs[et] = w2slab_t
            nc.sync.dma_start(
                out=w2slab_t[:].rearrange("p (a e) -> p a e", a=FT1),
                in_=w2_r[:, :, et * 128:(et + 1) * 128])

        sq2_tiles = [None] * ET
        for et in range(ET):
            w2slab = w2slabs[et % 2]
            for sh in range(SH):
                sl = slice(sh * 512, (sh + 1) * 512)
                ps = psD.tile([128, 512], F32, tag="psD")
                for ftk in range(FT1):
                    nc.tensor.matmul(
                        ps[:],
                        w2slab[:, ftk * 128:(ftk + 1) * 128],
                        z_sb[ftk][:, sl],
                        start=(ftk == 0), stop=(ftk == FT1 - 1))
                # y = ffn2 + b2 + h, in place: x tile becomes y
                nc.vector.scalar_tensor_tensor(
                    x_sb[et][:, sl], ps[:], b2s[:, et:et + 1],
                    hT_sb[et][:, sl], op0=OP.add, op1=OP.add)
            if et + 2 < ET:
                w2slab_t = w2p.tile([128, FT1 * 128], BF16, tag="w2slab",
                                    name=f"w2slab{et + 2}")
                w2slabs[et % 2] = w2slab_t
                nc.sync.dma_start(
                    out=w2slab_t[:].rearrange("p (a e) -> p a e", a=FT1),
                    in_=w2_r[:, :, (et + 2) * 128:(et + 3) * 128])
            if et > 0:
                ln_mms(et - 1, x_sb, sq2_tiles, ps2_sum, ps2_sq, ln2p)
        ln_mms(ET - 1, x_sb, sq2_tiles, ps2_sum, ps2_sq, ln2p)
        w2p.release()
        zp.release()
        psD.release()

        # LN2 in place: x tiles (y) -> out, DMA out per et as it lands
        def dma_out(et):
            nc.sync.dma_start(
                out=out_d[et * 128:(et + 1) * 128, :], in_=x_sb[et][:])

        ln_finalize(x_sb, x_sb, g2s, be2s, ps2_sum, ps2_sq, ln2p,
                    per_et_done=dma_out)
        psLN2.release()
        ln2p.release()
        xp.release()
        cpool.release()
    _split_multi_waits(nc)
    return nc


def prep_inputs(x, in_proj_w, out_proj_w, ln1_g, ln1_b, ln2_g, ln2_b,
                w1, b1, w2, b2):
    """Host-side reshapes/transposes. Returns (shared weight map, per-core xT)."""
    f32 = np.float32
    ET = E // 128

    def pcols(v, n):  # [n*128] vector -> [128, n] per-partition column layout
        return np.ascontiguousarray(np.asarray(v, f32).reshape(n, 128).T)

    shared = {
        "ones128": np.full((128, 128), 1.0 / E, f32),
        "onesr": np.ones((1, 64), f32),
        "wqkT": np.ascontiguousarray(np.asarray(in_proj_w, f32)[:2 * E].T),
        "wvT": np.ascontiguousarray(np.asarray(in_proj_w, f32)[2 * E:].T),
        "woT": np.ascontiguousarray(np.asarray(out_proj_w, f32).T),
        "w1T": np.ascontiguousarray(np.asarray(w1, f32).T),
        "w2T": np.ascontiguousarray(np.asarray(w2, f32).T).astype(
            ml_dtypes.bfloat16),
        "b1t": pcols(b1, FF // 128),
        "b2t": pcols(b2, ET),
        "g1t": pcols(ln1_g, ET),
        "be1t": pcols(ln1_b, ET),
        "g2t": pcols(ln2_g, ET),
        "be2t": pcols(ln2_b, ET),
    }
    x = np.asarray(x, f32)
    xTs = [np.ascontiguousarray(x[b].T) for b in range(x.shape[0])]
    return shared, xTs


def kernel(x, in_proj_w, out_proj_w, ln1_g, ln1_b, ln2_g, ln2_b,
           w1, b1, w2, b2, _trace=False, _cfg=None):
    S = x.shape[1]
    nc = build_bass(S=S, cfg=_cfg)
    shared, xTs = prep_inputs(x, in_proj_w, out_proj_w, ln1_g, ln1_b,
                              ln2_g, ln2_b, w1, b1, w2, b2)
    in_maps = [dict(shared, xT=xTs[b]) for b in range(x.shape[0])]
    res = run_bass_kernel_spmd(nc, in_maps, core_ids=list(range(NCORES)),
                               trace=_trace)
    out = np.stack([np.asarray(res.results[b]["outT"], np.float32).T
                    for b in range(x.shape[0])])
    if _trace:
        kernel.last_exec_time_ns = res.exec_time_ns
        kernel.last_results = res
    return out
